# revision 1
# baseline (speedup 1.0000x reference)
"""Trainium2 Bass kernel for nn_CRAU (per-channel sparse attention).

Computation (per batch b, channel c):
  qc  = Wq @ src (1x1 conv; bias folded into the exp bias)
  S[c,t] = sum_d unfold(qc)[c,t,d] * feat[c,d] * (1/64)      t in 3x3 window
  A   = softmax_t(S)
  vc  = Wv @ feat + bv (1x1 conv)
  out = fold(A outer vc) * src

Sharding: 8 cores = 4 batches x 2 output-channel halves. The attention is
fully per-channel, so with channel sharding each core is independent (no
collective). Each core needs all 256 input channels of src/feat for the
1x1 convs; the host permutes channels to [own 128 | other 128] so the
SPMD program is core-invariant, and the own-channel block doubles as the
k tensor and the final-multiply src.

Measured DVE rates (f16, aligned): tensor_scalar 4x (0.26 ns/elem),
tensor_tensor 2x (0.52), tensor_tensor_reduce / tensor_scalar+accum 1x
(1.07). Engine split per core:
 - 3 window taps run as single custom TENSOR_TENSOR_REDUCE ops on Vector;
   6 run as Vector tensor_tensor products + Scalar activation(Copy,
   scale=1/64, accum_out) reductions, balancing Vector vs Scalar.
 - exp(S_t + s_init_t) runs per tap on Scalar as soon as that tap lands;
   the unnormalized fold combines sum(E_t * vc_shift) run DURING the
   q-conv: multiplies on Vector tensor_scalar (4x), pair adds on GpSimd.
 - after the last tap only: sumE, 1/sumE, and per output parity plane
   (F~ * r) * src -- ~14 us of Vector tail.
The v-conv runs first on the PE so vc exists before the fold prework.

Layout: host-packed polyphase f16 planes of the padded 129x129 grid;
65-wide planes carry a leading zero pad column (width 66) so fold-stage
reads are 4-byte aligned. A column-shifted aligned shadow of vc (vcs)
keeps the v01/v11 reads aligned. Outputs are four 64x64 parity planes in
f16, re-interleaved on the host.

Plane layouts (rows x cols, * = zero pad col):
  EE [65 x 66] = [* | P[0::2, 0::2]]   (leading pad)
  EO [65 x 64] =      P[0::2, 1::2]
  OE [64 x 66] = [* | P[1::2, 0::2]]   (leading pad)
  OO [64 x 64] =      P[1::2, 1::2]
where P is the zero-padded [129,129] grid, P[r,c] = x[r-1, c-1].
"""

import numpy as np

N_CORES = 8
SCALE = 1.0 / 64.0

# plane offsets within the packed polyphase layout
P_EE, P_EO, P_OE, P_OO = 0, 4290, 8450, 12674
SRCN = 16770                    # 65*66 + 65*64 + 64*66 + 64*64
FEATN = 4290                    # 65*66 natural padded grid (2 pad cols)
OUTN = 16384                    # 4 planes x 64*64

_prog_cache = {}
TRACE = False
TRACE_KW = {}
LAST_RESULT = [None]
STAGE = [99]

# matmul / copy chunks (first small so the PE starts early)
CHUNKS = [1024] * 16 + [386]
EE_RDY, EO_RDY, OE_RDY, OO_RDY = 4, 8, 12, 16


def _build(add_bv: bool, stage: int = 99):
    import concourse.mybir as mybir
    import concourse.tile as tile
    from concourse import bacc
    from concourse.dve_ops import TENSOR_TENSOR_REDUCE

    f32 = mybir.dt.float32
    f16 = mybir.dt.float16
    ADD = mybir.AluOpType.add
    MULT = mybir.AluOpType.mult
    AX = mybir.AxisListType.X
    Exp = mybir.ActivationFunctionType.Exp
    Copy = mybir.ActivationFunctionType.Copy

    nc = bacc.Bacc("TRN2", target_bir_lowering=False, debug=False,
                   num_devices=N_CORES)

    src_d = nc.dram_tensor("src", [256, SRCN], f16, kind="ExternalInput").ap()
    feat_d = nc.dram_tensor("feat", [256, FEATN], f16,
                            kind="ExternalInput").ap()
    wpack_d = nc.dram_tensor("wpack", [256, 256], f16,
                             kind="ExternalInput").ap()
    sinit_d = nc.dram_tensor("s_init", [128, 9], f32,
                             kind="ExternalInput").ap()
    bv_d = nc.dram_tensor("bv", [128, 1], f32, kind="ExternalInput").ap()
    out_d = nc.dram_tensor("out", [128, OUTN], f16, kind="ExternalOutput").ap()

    coff = [0]
    for cs in CHUNKS:
        coff.append(coff[-1] + cs)

    with tile.TileContext(nc) as tc:
        with (
            tc.tile_pool(name="constp", bufs=2) as constp,
            tc.tile_pool(name="srcp", bufs=2) as srcp,
            tc.tile_pool(name="featp", bufs=2) as featp,
            tc.tile_pool(name="qcp", bufs=1) as qcp,
            tc.tile_pool(name="vcp", bufs=1) as vcp,
            tc.tile_pool(name="smp", bufs=1) as smp,
            tc.tile_pool(name="mp", bufs=1) as mp,
            tc.tile_pool(name="outp", bufs=2) as outp,
            tc.tile_pool(name="ps", bufs=4, space="PSUM") as ps,
        ):
            # smalls (cols): [0:9] S  [9:18] E  [27:28] sumE  [28:29] r
            # [32:41] s_init  [48:49] bv
            sm = smp.tile([128, 64], f32, tag="smalls")
            nc.sync.dma_start(sm[:, 32:41], sinit_d[:, :])
            if add_bv:
                nc.sync.dma_start(sm[:, 48:49], bv_d[:, :])

            # ---- loads ----
            w_t = []
            for kt in range(2):
                wt = constp.tile([128, 256], f16, tag="w")
                nc.sync.dma_start(wt[:], wpack_d[128 * kt:128 * kt + 128, :])
                w_t.append(wt)
            src_t = [srcp.tile([128, SRCN], f16, tag="src", name=f"src{k}")
                     for k in range(2)]
            feat_t = [featp.tile([128, FEATN], f16, tag="feat",
                                 name=f"feat{k}") for k in range(2)]
            nc.sync.dma_start(feat_t[0][:], feat_d[0:128, :])
            for kt in range(2):
                nc.sync.dma_start(src_t[kt][:, 0:coff[1]],
                                  src_d[128 * kt:128 * kt + 128, 0:coff[1]])
            for kt in range(2):
                nc.sync.dma_start(
                    src_t[kt][:, coff[1]:coff[2]],
                    src_d[128 * kt:128 * kt + 128, coff[1]:coff[2]])
            nc.sync.dma_start(feat_t[1][:], feat_d[128:256, :])
            c = 2
            while c < len(CHUNKS):
                c2 = min(c + 2, len(CHUNKS))
                for kt in range(2):
                    nc.sync.dma_start(
                        src_t[kt][:, coff[c]:coff[c2]],
                        src_d[128 * kt:128 * kt + 128, coff[c]:coff[c2]])
                c = c2

            qc = qcp.tile([128, SRCN], f16, tag="qc")
            qEE = qc[:, P_EE:P_EO].rearrange("p (r q) -> p r q", q=66)
            qEO = qc[:, P_EO:P_OE].rearrange("p (r q) -> p r q", q=64)
            qOE = qc[:, P_OE:P_OO].rearrange("p (r q) -> p r q", q=66)
            qOO = qc[:, P_OO:SRCN].rearrange("p (r q) -> p r q", q=64)
            kv = feat_t[0].rearrange("p (r q) -> p r q", q=66)[:, 0:64, 0:64]

            # fold-prework working tiles; mE/mF double as tap scratch
            mA = mp.tile([128, 4096], f16, tag="mA")
            mB = mp.tile([128, 4096], f16, tag="mB")
            mC = mp.tile([128, 4096], f16, tag="mC")
            mD = mp.tile([128, 4096], f16, tag="mD")
            mE = mp.tile([128, 4224], f16, tag="mE")
            mF = mp.tile([128, 4224], f16, tag="mF")

            def v2(tl):
                return tl.rearrange("p (r q) -> p r q", q=64)

            def e(t):
                return sm[:, 9 + t:10 + t]

            def tap_ttr(t, qview, scr):
                nc.vector._custom_dve(
                    TENSOR_TENSOR_REDUCE,
                    out=scr[:, 0:4096].rearrange("p (r q) -> p r q", q=64),
                    in0=qview, in1=kv, s0=0.0, s1=SCALE,
                    accum_out=sm[:, t:t + 1])
                nc.scalar.activation(e(t), sm[:, t:t + 1], Exp,
                                     bias=sm[:, 32 + t:33 + t], scale=1.0)

            def tap_split(t, qview, scr):
                s3 = scr[:, 0:4096].rearrange("p (r q) -> p r q", q=64)
                nc.vector.tensor_tensor(out=s3, in0=qview, in1=kv, op=MULT)
                nc.scalar.activation(scr[:, 0:4096], scr[:, 0:4096], Copy,
                                     bias=0.0, scale=SCALE,
                                     accum_out=sm[:, t:t + 1])
                nc.scalar.activation(e(t), sm[:, t:t + 1], Exp,
                                     bias=sm[:, 32 + t:33 + t], scale=1.0)

            def emit_vconv():
                vc = vcp.tile([128, FEATN], f16, tag="vc")
                for c0 in range(0, FEATN, 1024):
                    csz = min(1024, FEATN - c0)
                    pt = ps.tile([128, 1024], f32, tag="mm")
                    for kt in range(2):
                        for s0 in range(0, csz, 512):
                            ssz = min(512, csz - s0)
                            nc.tensor.matmul(
                                pt[:, s0:s0 + ssz],
                                lhsT=w_t[kt][:, 128:256],
                                rhs=feat_t[kt][:, c0 + s0:c0 + s0 + ssz],
                                start=(kt == 0), stop=(kt == 1))
                    if add_bv:
                        nc.vector.tensor_scalar(
                            out=vc[:, c0:c0 + csz], in0=pt[:, 0:csz],
                            scalar1=sm[:, 48:49], scalar2=None, op0=ADD)
                    else:
                        nc.scalar.copy(vc[:, c0:c0 + csz], pt[:, 0:csz])
                vc3 = vc.rearrange("p (r q) -> p r q", q=66)
                if add_bv:
                    nc.gpsimd.memset(vc3[:, 64, :], 0.0)
                    nc.gpsimd.memset(vc3[:, :, 64:66], 0.0)
                vcs = vcp.tile([128, 65 * 64], f16, tag="vcs")
                vcs3 = vcs.rearrange("p (r q) -> p r q", q=64)
                nc.vector.tensor_copy(vcs3[:], vc3[:, 0:65, 1:65])
                return vc3, vcs3

            vc3 = vcs3 = None
            views = {}
            for c, csz in enumerate(CHUNKS):
                c0 = coff[c]
                # v-conv + vcs after chunk 2's matmuls (feat arrived by then)
                if c == 5 and stage >= 4:
                    vc3, vcs3 = emit_vconv()
                    views = dict(v00=vc3[:, 0:64, 0:64],
                                 v10=vc3[:, 1:65, 0:64],
                                 v01=vcs3[:, 0:64, :],
                                 v11=vcs3[:, 1:65, :])
                pt = ps.tile([128, 1024], f32, tag="mm")
                for kt in range(2):
                    for s0 in range(0, csz, 512):
                        ssz = min(512, csz - s0)
                        nc.tensor.matmul(
                            pt[:, s0:s0 + ssz],
                            lhsT=w_t[kt][:, 0:128],
                            rhs=src_t[kt][:, c0 + s0:c0 + s0 + ssz],
                            start=(kt == 0), stop=(kt == 1))
                if stage < 2 and c > 0:
                    continue
                nc.scalar.copy(qc[:, c0:c0 + csz], pt[:, 0:csz])
                if stage < 2:
                    continue
                if c == EE_RDY:      # EE plane ready: taps t0,t6 (ttr) t2,t8 (split)
                    tap_ttr(0, qEE[:, 0:64, 1:65], mE)
                    tap_ttr(6, qEE[:, 1:65, 1:65], mE)
                    tap_split(2, qEE[:, 0:64, 2:66], mF)
                    tap_split(8, qEE[:, 1:65, 2:66], mE)
                if c == 5 and stage >= 5:
                    # oo prework: E{0,2,6,8} * v -> mA..mD; G pair-sums
                    nc.vector.tensor_scalar(out=v2(mA), in0=views['v11'],
                                            scalar1=e(0), scalar2=None,
                                            op0=MULT)
                    nc.vector.tensor_scalar(out=v2(mB), in0=views['v10'],
                                            scalar1=e(2), scalar2=None,
                                            op0=MULT)
                    nc.vector.tensor_scalar(out=v2(mC), in0=views['v01'],
                                            scalar1=e(6), scalar2=None,
                                            op0=MULT)
                    nc.vector.tensor_scalar(out=v2(mD), in0=views['v00'],
                                            scalar1=e(8), scalar2=None,
                                            op0=MULT)
                    nc.gpsimd.tensor_tensor(out=v2(mA), in0=v2(mA),
                                            in1=v2(mB), op=ADD)
                    nc.gpsimd.tensor_tensor(out=v2(mC), in0=v2(mC),
                                            in1=v2(mD), op=ADD)
                if c == EO_RDY:      # EO plane: t1 (ttr), t7 (split)
                    tap_ttr(1, qEO[:, 0:64, 0:64], mE)
                    tap_split(7, qEO[:, 1:65, 0:64], mF)
                    if stage >= 5:
                        # oe prework: mB = E1*v10, mD = E7*v00; G: mB += mD
                        nc.vector.tensor_scalar(out=v2(mB), in0=views['v10'],
                                                scalar1=e(1), scalar2=None,
                                                op0=MULT)
                        nc.vector.tensor_scalar(out=v2(mD), in0=views['v00'],
                                                scalar1=e(7), scalar2=None,
                                                op0=MULT)
                        nc.gpsimd.tensor_tensor(out=v2(mB), in0=v2(mB),
                                                in1=v2(mD), op=ADD)
                if c == OE_RDY:      # OE plane: t3, t5 (split)
                    tap_split(3, qOE[:, 0:64, 1:65], mF)
                    tap_split(5, qOE[:, 0:64, 2:66], mE)
                    if stage >= 5:
                        # eo prework: mD = E3*v01, mF = E5*v00; G: mD += mF
                        # (mC keeps the oo pair-sum until the tail add)
                        nc.vector.tensor_scalar(out=v2(mD), in0=views['v01'],
                                                scalar1=e(3), scalar2=None,
                                                op0=MULT)
                        nc.vector.tensor_scalar(out=v2(mF[:, 0:4096]),
                                                in0=views['v00'],
                                                scalar1=e(5), scalar2=None,
                                                op0=MULT)
                        nc.gpsimd.tensor_tensor(out=v2(mD), in0=v2(mD),
                                                in1=v2(mF[:, 0:4096]),
                                                op=ADD)
                if c == OO_RDY:      # OO plane: t4 (ttr, the gate)
                    tap_ttr(4, qOO[:, 0:64, 0:64], mE)

            if stage == 2:
                nc.sync.dma_start(out_d[:, 0:9], sm[:, 0:9])

            # ---- normalization ----
            if stage >= 3:
                nc.vector.tensor_reduce(sm[:, 27:28], sm[:, 9:18],
                                        axis=AX, op=ADD)
                nc.vector.reciprocal(sm[:, 28:29], sm[:, 27:28])
            if stage == 3:
                nc.sync.dma_start(out_d[:, 16:25], sm[:, 9:18])

            # ---- tail: (F~ * r) * src per parity plane ----
            if stage >= 5:
                r = sm[:, 28:29]
                s3 = src_t[0]
                sEE = s3[:, P_EE:P_EO].rearrange("p (r q) -> p r q", q=66)
                sEO = s3[:, P_EO:P_OE].rearrange("p (r q) -> p r q", q=64)
                sOE = s3[:, P_OE:P_OO].rearrange("p (r q) -> p r q", q=66)
                sOO = s3[:, P_OO:SRCN].rearrange("p (r q) -> p r q", q=64)

                # ee: mF = (E4*v00)*r in one 2-scalar op (mF free)
                nc.vector.tensor_scalar(out=v2(mF[:, 0:4096]),
                                        in0=views['v00'],
                                        scalar1=e(4), scalar2=r,
                                        op0=MULT, op1=MULT)
                Pee = outp.tile([128, 4096], f16, tag="O")
                nc.vector.tensor_tensor(out=v2(Pee), in0=v2(mF[:, 0:4096]),
                                        in1=sOO[:, 0:64, 0:64], op=MULT)
                nc.sync.dma_start(out_d[:, 0:4096], Pee[:])
                # oo: final pair-add here, then *r, *src
                nc.vector.tensor_tensor(out=v2(mA), in0=v2(mA),
                                        in1=v2(mC), op=ADD)
                nc.vector.tensor_scalar(out=v2(mA), in0=v2(mA), scalar1=r,
                                        scalar2=None, op0=MULT)
                Poo = outp.tile([128, 4096], f16, tag="O")
                nc.vector.tensor_tensor(out=v2(Poo), in0=v2(mA),
                                        in1=sEE[:, 1:65, 2:66], op=MULT)
                nc.sync.dma_start(out_d[:, 12288:16384], Poo[:])
                # oe (F~oe in mB) -> multiplies the EO src plane
                nc.vector.tensor_scalar(out=v2(mB), in0=v2(mB), scalar1=r,
                                        scalar2=None, op0=MULT)
                Poe = outp.tile([128, 4096], f16, tag="O")
                nc.vector.tensor_tensor(out=v2(Poe), in0=v2(mB),
                                        in1=sEO[:, 1:65, 0:64], op=MULT)
                nc.sync.dma_start(out_d[:, 8192:12288], Poe[:])
                # eo (F~eo in mD) -> multiplies the OE src plane
                nc.vector.tensor_scalar(out=v2(mD), in0=v2(mD), scalar1=r,
                                        scalar2=None, op0=MULT)
                Peo = outp.tile([128, 4096], f16, tag="O")
                nc.vector.tensor_tensor(out=v2(Peo), in0=v2(mD),
                                        in1=sOE[:, 0:64, 2:66], op=MULT)
                nc.sync.dma_start(out_d[:, 4096:8192], Peo[:])

    nc.compile()
    return nc


def _get_program(add_bv: bool, stage: int = 99):
    key = (add_bv, stage)
    if key not in _prog_cache:
        _prog_cache[key] = _build(add_bv, stage)
    return _prog_cache[key]


def _polyphase(x):
    # x: [B, C, 129, 129] padded f16 -> [B, C, 16770] plane-packed with
    # leading zero pad col on the 65-wide (even-col) planes
    B, C = x.shape[:2]
    ee = np.zeros((B, C, 65, 66), np.float16)
    ee[:, :, :, 1:66] = x[:, :, 0::2, 0::2]
    oe = np.zeros((B, C, 64, 66), np.float16)
    oe[:, :, :, 1:66] = x[:, :, 1::2, 0::2]
    return np.concatenate([
        ee.reshape(B, C, -1),
        x[:, :, 0::2, 1::2].reshape(B, C, -1),
        oe.reshape(B, C, -1),
        x[:, :, 1::2, 1::2].reshape(B, C, -1),
    ], axis=2)


def kernel(feat, src, Wq, bq, Wv, bv):
    from concourse.bass_utils import run_bass_kernel_spmd

    feat = np.asarray(feat, dtype=np.float32)
    src = np.asarray(src, dtype=np.float32)
    Wq = np.asarray(Wq, dtype=np.float32)
    bq = np.asarray(bq, dtype=np.float32)
    Wv = np.asarray(Wv, dtype=np.float32)
    bv = np.asarray(bv, dtype=np.float32)
    B, C, H, W = src.shape
    CH_HALF = C // 2

    # padded 129x129 grid (row/col -1 pad; right/bottom pad never read)
    src_pad = np.zeros((B, C, 129, 129), np.float16)
    src_pad[:, :, 1:129, 1:129] = src
    src_pk = _polyphase(src_pad)                       # [B, C, 16770]
    feat_pk = np.zeros((B, C, 65, 66), np.float16)
    feat_pk[:, :, 0:64, 0:64] = feat
    feat_pk = feat_pk.reshape(B, C, FEATN)

    add_bv = bool(np.any(bv))
    nc = _get_program(add_bv, STAGE[0])

    in_maps = []
    for core in range(N_CORES):
        b, u = core // 2, core % 2
        own = slice(CH_HALF * u, CH_HALF * u + CH_HALF)
        perm = np.r_[own, slice(CH_HALF * (1 - u), CH_HALF * (1 - u) + CH_HALF)]
        wpack = np.concatenate(
            [Wq[own][:, perm].T, Wv[own][:, perm].T], axis=1
        ).astype(np.float16)
        # bq correction seed: S += bq * sum(valid k) * scale; valid excludes
        # x=0 when i==0 and y=0 when j==0 (qc zero-pad positions).
        if np.any(bq):
            k = feat[b, own].astype(np.float64)
            tot = k.sum((1, 2))
            no_r0 = tot - k[:, 0, :].sum(1)
            no_c0 = tot - k[:, :, 0].sum(1)
            no_rc = no_r0 - k[:, :, 0].sum(1) + k[:, 0, 0]
            sums = [no_rc, no_r0, no_r0, no_c0, tot, tot, no_c0, tot, tot]
            sinit = (np.stack(sums, 1) * bq[own, None] * SCALE).astype(
                np.float32)
        else:
            sinit = np.zeros((CH_HALF, 9), np.float32)
        in_maps.append({
            "src": np.ascontiguousarray(src_pk[b, perm]),
            "feat": np.ascontiguousarray(feat_pk[b, perm]),
            "wpack": np.ascontiguousarray(wpack),
            "s_init": sinit,
            "bv": bv[own].reshape(CH_HALF, 1).astype(np.float32),
        })

    res = run_bass_kernel_spmd(nc, in_maps, list(range(N_CORES)),
                               trace=TRACE, **TRACE_KW)
    LAST_RESULT[0] = res

    out = np.empty((B, C, H, W), np.float32)
    for core in range(N_CORES):
        b, u = core // 2, core % 2
        own = slice(CH_HALF * u, CH_HALF * u + CH_HALF)
        r = res.results[core]["out"].astype(np.float32).reshape(
            CH_HALF, 4, 64, 64)
        out[b, own, 0::2, 0::2] = r[:, 0]
        out[b, own, 0::2, 1::2] = r[:, 1]
        out[b, own, 1::2, 0::2] = r[:, 2]
        out[b, own, 1::2, 1::2] = r[:, 3]
    return out



# revision 10
# speedup vs baseline: 1.0294x; 1.0294x over previous
"""Trainium2 Bass kernel for nn_CRAU (per-channel sparse attention).

Computation (per batch b, channel c):
  qc  = Wq @ src (1x1 conv)
  S[c,t] = sum_d unfold(qc)[c,t,d] * feat[c,d] * (1/64)      t in 3x3 window
  A   = softmax_t(S);  vc = Wv @ feat + bv
  out = fold(A outer vc) * src

Sharding: 8 cores = 4 batches x 2 output-channel halves (no collectives;
each core loads the full-batch src/feat for the 1x1 conv contraction,
host permutes channels to [own 128 | other 128]).

v2 engine split (per core), from measured DVE/ACT/PE rates:
 - taps: Vector TENSOR_TENSOR products (2x mode) + Scalar activation
   accumulate; two early taps as Vector TTR (1x) to balance engines.
 - fold: TensorE diagonal matmuls -- diag(E_t) built on Vector from an
   uploaded identity, PSUM-accumulated per parity half-plane; Vector TT
   multiplies PSUM F~ by the src plane directly (f32 PSUM operand, 1x).
 - the softmax normalizer r=1/sumE is applied AFTER (F~ . src) as a 4x
   tensor_scalar per plane, so only ~5us of work serializes behind the
   last tap.
 - qc PSUM->SBUF f16 copies and the v-conv evacuation run on Scalar.
 - GpSimd does nothing bulk (shares an SBUF port with Vector).

Plane layouts as v1: polyphase packed padded 129x129 grid, planes
  EE [65x66, lead pad col] | EO [65x64] | OE [64x66, lead pad] | OO [64x64]
qc plane -> taps: EE {0,2,6,8}, EO {1,7}, OE {3,5}, OO {4}.
out plane <- taps: ee {4}, oo {0,2,6,8}, oe {1,7}, eo {3,5}.
Accumulator slot order (host reorders s_init to match):
  [t0,t2,t6,t8, t1,t7, t3,t5, t4]
"""

import numpy as np

N_CORES = 8
SCALE = 1.0 / 64.0

P_EE, P_EO, P_OE, P_OO = 0, 4290, 8450, 12674
SRCN = 16770
FEATN = 4290
OUTN = 16384

_prog_cache = {}
TRACE = False
TRACE_KW = {}
LAST_RESULT = [None]

CHUNKS = [1024] * 16 + [386]
EE_RDY, EO_RDY, OE_RDY = 4, 8, 12
# tap -> accumulator slot (grouped by qc plane for one-exp-per-plane)
SLOT = {0: 0, 2: 1, 6: 2, 8: 3, 1: 4, 7: 5, 3: 6, 5: 7, 4: 8}
SLOT_ORDER = [0, 2, 6, 8, 1, 7, 3, 5, 4]
TTR_TAPS = (0, 6)               # run as Vector TTR; rest TT + Scalar accum


def _build(add_bv: bool, per_tap_bias: bool):
    import concourse.mybir as mybir
    import concourse.tile as tile
    from concourse import bacc
    from concourse.dve_ops import TENSOR_TENSOR_REDUCE

    f32 = mybir.dt.float32
    f16 = mybir.dt.float16
    ADD = mybir.AluOpType.add
    MULT = mybir.AluOpType.mult
    AX = mybir.AxisListType.X
    Exp = mybir.ActivationFunctionType.Exp
    Copy = mybir.ActivationFunctionType.Copy

    nc = bacc.Bacc("TRN2", target_bir_lowering=False, debug=False,
                   num_devices=N_CORES)

    src_d = nc.dram_tensor("src", [256, SRCN], f16, kind="ExternalInput").ap()
    feat_d = nc.dram_tensor("feat", [256, FEATN], f16,
                            kind="ExternalInput").ap()
    wpack_d = nc.dram_tensor("wpack", [256, 256], f16,
                             kind="ExternalInput").ap()
    sinit_d = nc.dram_tensor("s_init", [128, 9], f32,
                             kind="ExternalInput").ap()
    bv_d = nc.dram_tensor("bv", [128, 1], f32, kind="ExternalInput").ap()
    ident_d = nc.dram_tensor("ident", [128, 128], f16,
                             kind="ExternalInput").ap()
    out_d = nc.dram_tensor("out", [128, OUTN], f16, kind="ExternalOutput").ap()

    coff = [0]
    for cs in CHUNKS:
        coff.append(coff[-1] + cs)

    with tile.TileContext(nc) as tc:
        with (
            tc.tile_pool(name="constp", bufs=2) as constp,
            tc.tile_pool(name="srcp", bufs=2) as srcp,
            tc.tile_pool(name="featp", bufs=2) as featp,
            tc.tile_pool(name="qcp", bufs=1) as qcp,
            tc.tile_pool(name="vcp", bufs=1) as vcp,
            tc.tile_pool(name="smp", bufs=1) as smp,
            tc.tile_pool(name="prodp", bufs=2) as prodp,
            tc.tile_pool(name="outp", bufs=4) as outp,
            tc.tile_pool(name="ps", bufs=2, space="PSUM") as ps,
            tc.tile_pool(name="fps", bufs=1, space="PSUM") as fps,
        ):
            # smalls (cols): [0:9] S by slot  [9:18] E by slot  [27] sumE
            # [28] r  [32:41] s_init by slot  [48] bv
            sm = smp.tile([128, 64], f32, tag="smalls")
            nc.sync.dma_start(sm[:, 32:41], sinit_d[:, :])
            if add_bv:
                nc.sync.dma_start(sm[:, 48:49], bv_d[:, :])

            # ---- loads: w, ident, feat (k + v-conv input), then src ----
            w_t = []
            for kt in range(2):
                wt = constp.tile([128, 256], f16, tag="w")
                nc.sync.dma_start(wt[:], wpack_d[128 * kt:128 * kt + 128, :])
                w_t.append(wt)
            ident = constp.tile([128, 128], f16, tag="ident")
            nc.sync.dma_start(ident[:], ident_d[:, :])

            feat_t = [featp.tile([128, FEATN], f16, tag="feat",
                                 name=f"feat{k}") for k in range(2)]
            nc.sync.dma_start(feat_t[0][:], feat_d[0:128, :])

            src_t = [srcp.tile([128, SRCN], f16, tag="src", name=f"src{k}")
                     for k in range(2)]
            for c in range(len(CHUNKS)):
                if c == 5:
                    nc.sync.dma_start(feat_t[1][:], feat_d[128:256, :])
                for kt in range(2):
                    nc.sync.dma_start(
                        src_t[kt][:, coff[c]:coff[c + 1]],
                        src_d[128 * kt:128 * kt + 128, coff[c]:coff[c + 1]])

            qc = qcp.tile([128, SRCN], f16, tag="qc")
            qEE = qc[:, P_EE:P_EO].rearrange("p (r q) -> p r q", q=66)
            qEO = qc[:, P_EO:P_OE].rearrange("p (r q) -> p r q", q=64)
            qOE = qc[:, P_OE:P_OO].rearrange("p (r q) -> p r q", q=66)
            qOO = qc[:, P_OO:SRCN].rearrange("p (r q) -> p r q", q=64)
            kv = feat_t[0].rearrange("p (r q) -> p r q", q=66)[:, 0:64, 0:64]

            # product scratch + diag bank
            pr = [prodp.tile([128, 4096], f16, tag="prod", name=f"pr{k}")
                  for k in range(2)]
            dg = constp.tile([128, 9 * 128], f16, tag="diag")

            def tap(t, qview, scr):
                sl = SLOT[t]
                if t in TTR_TAPS:
                    nc.vector._custom_dve(
                        TENSOR_TENSOR_REDUCE,
                        out=scr[:, 0:4096].rearrange("p (r q) -> p r q", q=64),
                        in0=qview, in1=kv, s0=0.0, s1=SCALE,
                        accum_out=sm[:, sl:sl + 1])
                else:
                    s3 = scr[:, 0:4096].rearrange("p (r q) -> p r q", q=64)
                    nc.vector.tensor_tensor(out=s3, in0=qview, in1=kv, op=MULT)
                    nc.scalar.activation(scr[:, 0:4096], scr[:, 0:4096], Copy,
                                         bias=0.0, scale=SCALE,
                                         accum_out=sm[:, sl:sl + 1])

            def exp_group(sl0, sl1):
                # E[slots] = exp(S[slots] + s_init[slots])
                if per_tap_bias:
                    for sl in range(sl0, sl1):
                        nc.scalar.activation(sm[:, 9 + sl:10 + sl],
                                             sm[:, sl:sl + 1], Exp,
                                             bias=sm[:, 32 + sl:33 + sl],
                                             scale=1.0)
                else:
                    nc.scalar.activation(sm[:, 9 + sl0:9 + sl1],
                                         sm[:, sl0:sl1], Exp,
                                         bias=0.0, scale=1.0)

            def diag(sl):
                nc.vector.tensor_scalar(
                    out=dg[:, sl * 128:sl * 128 + 128], in0=ident[:],
                    scalar1=sm[:, 9 + sl:10 + sl], scalar2=None, op0=MULT)

            def emit_vconv():
                vc = vcp.tile([128, FEATN], f16, tag="vc")
                for c0 in range(0, FEATN, 1024):
                    csz = min(1024, FEATN - c0)
                    pt = ps.tile([128, 1024], f32, tag="mm")
                    for kt in range(2):
                        for s0 in range(0, csz, 512):
                            ssz = min(512, csz - s0)
                            nc.tensor.matmul(
                                pt[:, s0:s0 + ssz],
                                lhsT=w_t[kt][:, 128:256],
                                rhs=feat_t[kt][:, c0 + s0:c0 + s0 + ssz],
                                start=(kt == 0), stop=(kt == 1))
                    if add_bv:
                        nc.vector.tensor_scalar(
                            out=vc[:, c0:c0 + csz], in0=pt[:, 0:csz],
                            scalar1=sm[:, 48:49], scalar2=None, op0=ADD)
                    else:
                        nc.scalar.copy(vc[:, c0:c0 + csz], pt[:, 0:csz])
                vc3 = vc.rearrange("p (r q) -> p r q", q=66)
                if add_bv:
                    nc.gpsimd.memset(vc3[:, 64, :], 0.0)
                    nc.gpsimd.memset(vc3[:, :, 64:66], 0.0)
                return vc3

            vc3 = None
            views = {}

            # ---- q-conv chunk loop with interleaved taps/folds ----
            # fold plane -> (tap slots, vc view keys); emitted at chunk idx
            def fold_plane(slots, vkeys, out_tile, srcv3):
                # PSUM-accumulated diag matmuls, one 2048-col half at a time;
                # Vector TT multiplies PSUM F~ by the src half directly.
                for h in range(2):
                    ft = fps.tile([128, 2048], f32, tag="fold")
                    f3 = ft.rearrange("p (r q) -> p r q", q=64)
                    for i, (sl, vk) in enumerate(zip(slots, vkeys)):
                        v3 = views[vk]
                        for b in range(4):
                            r0 = h * 32 + b * 8
                            nc.tensor.matmul(
                                f3[:, b * 8:b * 8 + 8, :],
                                lhsT=dg[:, sl * 128:sl * 128 + 128],
                                rhs=v3[:, r0:r0 + 8, :],
                                start=(i == 0), stop=(i == len(slots) - 1))
                    nc.vector.tensor_tensor(
                        out=out_tile[:, h * 2048:h * 2048 + 2048]
                            .rearrange("p (r q) -> p r q", q=64),
                        in0=f3[:], in1=srcv3[:, h * 32:h * 32 + 32, :],
                        op=MULT)

            s3 = src_t[0]
            sEE = s3[:, P_EE:P_EO].rearrange("p (r q) -> p r q", q=66)
            sEO = s3[:, P_EO:P_OE].rearrange("p (r q) -> p r q", q=64)
            sOE = s3[:, P_OE:P_OO].rearrange("p (r q) -> p r q", q=66)
            sOO = s3[:, P_OO:SRCN].rearrange("p (r q) -> p r q", q=64)

            oEE = outp.tile([128, 4096], f16, tag="O", name="oEE")
            oEO = outp.tile([128, 4096], f16, tag="O", name="oEO")
            oOE = outp.tile([128, 4096], f16, tag="O", name="oOE")
            oOO = outp.tile([128, 4096], f16, tag="O", name="oOO")

            for c, csz in enumerate(CHUNKS):
                c0 = coff[c]
                if c == 10:
                    vc3 = emit_vconv()
                    views = dict(v00=vc3[:, 0:64, 0:64],
                                 v10=vc3[:, 1:65, 0:64],
                                 v01=vc3[:, 0:64, 1:65],
                                 v11=vc3[:, 1:65, 1:65])
                pt = ps.tile([128, 1024], f32, tag="mm")
                for kt in range(2):
                    for s0 in range(0, csz, 512):
                        ssz = min(512, csz - s0)
                        nc.tensor.matmul(
                            pt[:, s0:s0 + ssz],
                            lhsT=w_t[kt][:, 0:128],
                            rhs=src_t[kt][:, c0 + s0:c0 + s0 + ssz],
                            start=(kt == 0), stop=(kt == 1))
                nc.scalar.copy(qc[:, c0:c0 + csz], pt[:, 0:csz])

                if c == EE_RDY:
                    tap(0, qEE[:, 0:64, 1:65], pr[0])
                    tap(6, qEE[:, 1:65, 1:65], pr[1])
                    tap(2, qEE[:, 0:64, 2:66], pr[0])
                    tap(8, qEE[:, 1:65, 2:66], pr[1])
                    exp_group(0, 4)
                    for sl in range(4):
                        diag(sl)
                if c == EO_RDY:
                    tap(1, qEO[:, 0:64, 0:64], pr[0])
                    tap(7, qEO[:, 1:65, 0:64], pr[1])
                    exp_group(4, 6)
                    diag(4)
                    diag(5)
                if c == OE_RDY:
                    tap(3, qOE[:, 0:64, 1:65], pr[0])
                    tap(5, qOE[:, 0:64, 2:66], pr[1])
                    exp_group(6, 8)
                    diag(6)
                    diag(7)
                if c == 13:
                    # oo: F~ = E0*v11 + E2*v10 + E6*v01 + E8*v00, times sEE
                    fold_plane([0, 1, 2, 3], ['v11', 'v10', 'v01', 'v00'],
                               oOO, sEE[:, 1:65, 2:66])
                if c == 14:
                    # tap4 rows 0:32 (cols ready by c14: 12674+2048=14722)
                    nc.vector.tensor_tensor(
                        out=pr[0][:, 0:2048].rearrange("p (r q) -> p r q",
                                                       q=64),
                        in0=qOO[:, 0:32, 0:64], in1=kv[:, 0:32, :], op=MULT)
                    nc.scalar.activation(pr[0][:, 0:2048], pr[0][:, 0:2048],
                                         Copy, bias=0.0, scale=SCALE,
                                         accum_out=sm[:, 24:25])
                if c == 15:
                    # oe: F~ = E1*v10 + E7*v00, times sEO
                    fold_plane([4, 5], ['v10', 'v00'], oOE,
                               sEO[:, 1:65, 0:64])

            # eo: F~ = E3*v01 + E5*v00, times sOE
            fold_plane([6, 7], ['v01', 'v00'], oEO, sOE[:, 0:64, 2:66])

            # tap4 rows 32:64 after the last chunk
            nc.vector.tensor_tensor(
                out=pr[1][:, 0:2048].rearrange("p (r q) -> p r q", q=64),
                in0=qOO[:, 32:64, 0:64], in1=kv[:, 32:64, :], op=MULT)
            nc.scalar.activation(pr[1][:, 0:2048], pr[1][:, 0:2048],
                                 Copy, bias=0.0, scale=SCALE,
                                 accum_out=sm[:, 25:26])
            nc.vector.tensor_tensor(out=sm[:, 8:9], in0=sm[:, 24:25],
                                    in1=sm[:, 25:26], op=ADD)
            exp_group(8, 9)

            # ---- normalization ----
            nc.vector.tensor_reduce(sm[:, 27:28], sm[:, 9:18],
                                    axis=AX, op=ADD)
            nc.vector.reciprocal(sm[:, 28:29], sm[:, 27:28])
            r = sm[:, 28:29]

            # ---- tail ----
            # ee: (E4*r*v00) . sOO  (two fused-scalar TS + TT)
            nc.vector.tensor_scalar(
                out=pr[0][:, 0:4096].rearrange("p (r q) -> p r q", q=64),
                in0=views['v00'], scalar1=sm[:, 17:18], scalar2=r,
                op0=MULT, op1=MULT)
            nc.vector.tensor_tensor(
                out=oEE.rearrange("p (r q) -> p r q", q=64),
                in0=pr[0][:, 0:4096].rearrange("p (r q) -> p r q", q=64),
                in1=sOO[:, 0:64, 0:64], op=MULT)
            nc.sync.dma_start(out_d[:, 0:4096], oEE[:])
            # remaining planes: out *= r (4x-mode tensor_scalar), then DMA
            nc.vector.tensor_scalar(out=oOO[:], in0=oOO[:], scalar1=r,
                                    scalar2=None, op0=MULT)
            nc.sync.dma_start(out_d[:, 12288:16384], oOO[:])
            nc.vector.tensor_scalar(out=oOE[:], in0=oOE[:], scalar1=r,
                                    scalar2=None, op0=MULT)
            nc.sync.dma_start(out_d[:, 8192:12288], oOE[:])
            nc.vector.tensor_scalar(out=oEO[:], in0=oEO[:], scalar1=r,
                                    scalar2=None, op0=MULT)
            nc.sync.dma_start(out_d[:, 4096:8192], oEO[:])

    nc.compile()
    return nc


def _get_program(add_bv: bool, per_tap_bias: bool):
    key = (add_bv, per_tap_bias)
    if key not in _prog_cache:
        _prog_cache[key] = _build(add_bv, per_tap_bias)
    return _prog_cache[key]


def _polyphase(x):
    B, C = x.shape[:2]
    ee = np.zeros((B, C, 65, 66), np.float16)
    ee[:, :, :, 1:66] = x[:, :, 0::2, 0::2]
    oe = np.zeros((B, C, 64, 66), np.float16)
    oe[:, :, :, 1:66] = x[:, :, 1::2, 0::2]
    return np.concatenate([
        ee.reshape(B, C, -1),
        x[:, :, 0::2, 1::2].reshape(B, C, -1),
        oe.reshape(B, C, -1),
        x[:, :, 1::2, 1::2].reshape(B, C, -1),
    ], axis=2)


def kernel(feat, src, Wq, bq, Wv, bv):
    from concourse.bass_utils import run_bass_kernel_spmd

    feat = np.asarray(feat, dtype=np.float32)
    src = np.asarray(src, dtype=np.float32)
    Wq = np.asarray(Wq, dtype=np.float32)
    bq = np.asarray(bq, dtype=np.float32)
    Wv = np.asarray(Wv, dtype=np.float32)
    bv = np.asarray(bv, dtype=np.float32)
    B, C, H, W = src.shape
    CH_HALF = C // 2

    src_pad = np.zeros((B, C, 129, 129), np.float16)
    src_pad[:, :, 1:129, 1:129] = src
    src_pk = _polyphase(src_pad)                       # [B, C, 16770]
    feat_pk = np.zeros((B, C, 65, 66), np.float16)
    feat_pk[:, :, 0:64, 0:64] = feat
    feat_pk = feat_pk.reshape(B, C, FEATN)

    add_bv = bool(np.any(bv))
    per_tap_bias = bool(np.any(bq))
    nc = _get_program(add_bv, per_tap_bias)
    ident = np.eye(128, dtype=np.float16)

    in_maps = []
    for core in range(N_CORES):
        b, u = core // 2, core % 2
        own = slice(CH_HALF * u, CH_HALF * u + CH_HALF)
        perm = np.r_[own, slice(CH_HALF * (1 - u), CH_HALF * (1 - u) + CH_HALF)]
        wpack = np.concatenate(
            [Wq[own][:, perm].T, Wv[own][:, perm].T], axis=1
        ).astype(np.float16)
        if per_tap_bias:
            k = feat[b, own].astype(np.float64)
            tot = k.sum((1, 2))
            no_r0 = tot - k[:, 0, :].sum(1)
            no_c0 = tot - k[:, :, 0].sum(1)
            no_rc = no_r0 - k[:, :, 0].sum(1) + k[:, 0, 0]
            sums = [no_rc, no_r0, no_r0, no_c0, tot, tot, no_c0, tot, tot]
            sinit_t = (np.stack(sums, 1) * bq[own, None] * SCALE).astype(
                np.float32)
            sinit = sinit_t[:, SLOT_ORDER]
        else:
            sinit = np.zeros((CH_HALF, 9), np.float32)
        in_maps.append({
            "src": np.ascontiguousarray(src_pk[b, perm]),
            "feat": np.ascontiguousarray(feat_pk[b, perm]),
            "wpack": np.ascontiguousarray(wpack),
            "s_init": sinit,
            "bv": bv[own].reshape(CH_HALF, 1).astype(np.float32),
            "ident": ident,
        })

    res = run_bass_kernel_spmd(nc, in_maps, list(range(N_CORES)),
                               trace=TRACE, **TRACE_KW)
    LAST_RESULT[0] = res

    out = np.empty((B, C, H, W), np.float32)
    for core in range(N_CORES):
        b, u = core // 2, core % 2
        own = slice(CH_HALF * u, CH_HALF * u + CH_HALF)
        r = res.results[core]["out"].astype(np.float32).reshape(
            CH_HALF, 4, 64, 64)
        out[b, own, 0::2, 0::2] = r[:, 0]
        out[b, own, 0::2, 1::2] = r[:, 1]
        out[b, own, 1::2, 0::2] = r[:, 2]
        out[b, own, 1::2, 1::2] = r[:, 3]
    return out


# revision 15
# speedup vs baseline: 1.2644x; 1.2283x over previous
"""Trainium2 Bass kernel for nn_CRAU (per-channel sparse attention).

Computation (per batch b, channel c):
  qc  = Wq @ src (1x1 conv)
  S[c,t] = sum_d unfold(qc)[c,t,d] * feat[c,d] * (1/64)      t in 3x3 window
  A   = softmax_t(S);  vc = Wv @ feat + bv
  out = fold(A outer vc) * src

Sharding: 8 cores = 4 batches x 2 output-channel halves (no collectives).

v3 schedule (from v2 trace post-mortem): the Scalar FIFO must never
sit between the PE and its PSUM evacuation, so
 - qc chunks are plane-aligned <=2048 cols; one shared PSUM pool
   [128,2048]x2 rotates q-conv chunks, the v-conv, and the folds.
 - qc PSUM->SBUF copies all on Scalar, emitted immediately per chunk;
   tap accumulates are DEFERRED and interleaved one-per-chunk-copy so
   the copy stream stays <=1 chunk behind DMA.
 - taps t0,t6 and the two t4 halves run as Vector TTR (no Scalar);
   t2,t8,t1,t7,t3,t5 as Vector TT + deferred Scalar accumulate.
 - folds on TensorE as diag(E_t) matmuls (unnormalized E, PSUM f32);
   Vector TT multiplies PSUM directly by the src plane; 1/sumE is
   applied after as 4x-mode tensor_scalars.
Plane layouts as v1/v2 (polyphase packed padded 129x129 grid).
Accumulator slot order: [t0,t2,t6,t8, t1,t7, t3,t5, t4].
"""

import numpy as np

N_CORES = 8
SCALE = 1.0 / 64.0

P_EE, P_EO, P_OE, P_OO = 0, 4290, 8450, 12674
SRCN = 16770
FEATN = 4290
OUTN = 16384

_prog_cache = {}
TRACE = False
TRACE_KW = {}
LAST_RESULT = [None]

# plane-aligned chunks: EE 4290 | EO 4160 | OE 4224 | OO 4096
CHUNKS = [2048, 2048, 194, 2048, 2048, 64, 2048, 2048, 128, 2048, 2048]
EE_RDY, EO_RDY, OE_RDY = 2, 5, 8
SLOT = {0: 0, 2: 1, 6: 2, 8: 3, 1: 4, 7: 5, 3: 6, 5: 7, 4: 8}
SLOT_ORDER = [0, 2, 6, 8, 1, 7, 3, 5, 4]


def _build(add_bv: bool, per_tap_bias: bool):
    import concourse.mybir as mybir
    import concourse.tile as tile
    from concourse import bacc
    from concourse.dve_ops import TENSOR_TENSOR_REDUCE

    f32 = mybir.dt.float32
    f16 = mybir.dt.float16
    ADD = mybir.AluOpType.add
    MULT = mybir.AluOpType.mult
    AX = mybir.AxisListType.X
    Exp = mybir.ActivationFunctionType.Exp
    Copy = mybir.ActivationFunctionType.Copy

    nc = bacc.Bacc("TRN2", target_bir_lowering=False, debug=False,
                   num_devices=N_CORES)

    src_d = nc.dram_tensor("src", [256, SRCN], f16, kind="ExternalInput").ap()
    feat_d = nc.dram_tensor("feat", [256, FEATN], f16,
                            kind="ExternalInput").ap()
    wpack_d = nc.dram_tensor("wpack", [256, 256], f16,
                             kind="ExternalInput").ap()
    sinit_d = nc.dram_tensor("s_init", [128, 9], f32,
                             kind="ExternalInput").ap()
    bv_d = nc.dram_tensor("bv", [128, 1], f32, kind="ExternalInput").ap()
    ident_d = nc.dram_tensor("ident", [128, 128], f16,
                             kind="ExternalInput").ap()
    out_d = nc.dram_tensor("out", [128, OUTN], f16, kind="ExternalOutput").ap()

    coff = [0]
    for cs in CHUNKS:
        coff.append(coff[-1] + cs)

    with tile.TileContext(nc) as tc:
        with (
            tc.tile_pool(name="constp", bufs=2) as constp,
            tc.tile_pool(name="srcp", bufs=2) as srcp,
            tc.tile_pool(name="featp", bufs=2) as featp,
            tc.tile_pool(name="qcp", bufs=1) as qcp,
            tc.tile_pool(name="vcp", bufs=1) as vcp,
            tc.tile_pool(name="smp", bufs=1) as smp,
            tc.tile_pool(name="prodp", bufs=2) as prodp,
            tc.tile_pool(name="outp", bufs=4) as outp,
            tc.tile_pool(name="ps", bufs=2, space="PSUM") as ps,
        ):
            # smalls: [0:9] S by slot [9:18] E [24,25] t4 halves [27] sumE
            # [28] r [32:41] s_init [48] bv
            sm = smp.tile([128, 64], f32, tag="smalls")
            nc.sync.dma_start(sm[:, 32:41], sinit_d[:, :])
            if add_bv:
                nc.sync.dma_start(sm[:, 48:49], bv_d[:, :])

            w_t = []
            for kt in range(2):
                wt = constp.tile([128, 256], f16, tag="w")
                nc.sync.dma_start(wt[:], wpack_d[128 * kt:128 * kt + 128, :])
                w_t.append(wt)
            ident = constp.tile([128, 128], f16, tag="ident")
            nc.sync.dma_start(ident[:], ident_d[:, :])

            feat_t = [featp.tile([128, FEATN], f16, tag="feat",
                                 name=f"feat{k}") for k in range(2)]
            nc.sync.dma_start(feat_t[0][:], feat_d[0:128, :])

            src_t = [srcp.tile([128, SRCN], f16, tag="src", name=f"src{k}")
                     for k in range(2)]
            for c in range(len(CHUNKS)):
                if c == 3:
                    nc.sync.dma_start(feat_t[1][:], feat_d[128:256, :])
                for kt in range(2):
                    nc.sync.dma_start(
                        src_t[kt][:, coff[c]:coff[c + 1]],
                        src_d[128 * kt:128 * kt + 128, coff[c]:coff[c + 1]])

            qc = qcp.tile([128, SRCN], f16, tag="qc")
            qEE = qc[:, P_EE:P_EO].rearrange("p (r q) -> p r q", q=66)
            qEO = qc[:, P_EO:P_OE].rearrange("p (r q) -> p r q", q=64)
            qOE = qc[:, P_OE:P_OO].rearrange("p (r q) -> p r q", q=66)
            qOO = qc[:, P_OO:SRCN].rearrange("p (r q) -> p r q", q=64)
            kv = feat_t[0].rearrange("p (r q) -> p r q", q=66)[:, 0:64, 0:64]

            pr = [prodp.tile([128, 4096], f16, tag="prod", name=f"pr{k}")
                  for k in range(2)]
            dg = constp.tile([128, 9 * 128], f16, tag="diag")

            pend_s = []            # deferred Scalar ops, one per chunk copy

            def tap_ttr(t, qview, scr, sl=None, kview=None):
                sl = SLOT[t] if sl is None else sl
                kview = kv if kview is None else kview
                n = 1
                for d in qview.shape[1:]:
                    n *= d
                nc.vector._custom_dve(
                    TENSOR_TENSOR_REDUCE,
                    out=scr[:, 0:n].rearrange(
                        "p (r q) -> p r q", q=qview.shape[-1]),
                    in0=qview, in1=kview, s0=0.0,
                    s1=SCALE, accum_out=sm[:, sl:sl + 1])

            def tap_tt(t, qview, scr):
                sl = SLOT[t]
                s3 = scr[:, 0:4096].rearrange("p (r q) -> p r q", q=64)
                nc.vector.tensor_tensor(out=s3, in0=qview, in1=kv, op=MULT)

                def acc(scr=scr, sl=sl):
                    nc.scalar.activation(scr[:, 0:4096], scr[:, 0:4096],
                                         Copy, bias=0.0, scale=SCALE,
                                         accum_out=sm[:, sl:sl + 1])
                pend_s.append(acc)

            def exp_group(sl0, sl1):
                if per_tap_bias:
                    for sl in range(sl0, sl1):
                        nc.scalar.activation(sm[:, 9 + sl:10 + sl],
                                             sm[:, sl:sl + 1], Exp,
                                             bias=sm[:, 32 + sl:33 + sl],
                                             scale=1.0)
                else:
                    nc.scalar.activation(sm[:, 9 + sl0:9 + sl1],
                                         sm[:, sl0:sl1], Exp,
                                         bias=0.0, scale=1.0)

            def diag(sl):
                nc.vector.tensor_scalar(
                    out=dg[:, sl * 128:sl * 128 + 128], in0=ident[:],
                    scalar1=sm[:, 9 + sl:10 + sl], scalar2=None, op0=MULT)

            def emit_vconv():
                vc = vcp.tile([128, FEATN], f16, tag="vc")
                for c0 in (0, 2048, 4096):
                    csz = min(2048, FEATN - c0)
                    pt = ps.tile([128, 2048], f32, tag="mm")
                    for kt in range(2):
                        for s0 in range(0, csz, 512):
                            ssz = min(512, csz - s0)
                            nc.tensor.matmul(
                                pt[:, s0:s0 + ssz],
                                lhsT=w_t[kt][:, 128:256],
                                rhs=feat_t[kt][:, c0 + s0:c0 + s0 + ssz],
                                start=(kt == 0), stop=(kt == 1))
                    if add_bv:
                        nc.vector.tensor_scalar(
                            out=vc[:, c0:c0 + csz], in0=pt[:, 0:csz],
                            scalar1=sm[:, 48:49], scalar2=None, op0=ADD)
                    else:
                        nc.scalar.copy(vc[:, c0:c0 + csz], pt[:, 0:csz])
                vc3 = vc.rearrange("p (r q) -> p r q", q=66)
                if add_bv:
                    nc.gpsimd.memset(vc3[:, 64, :], 0.0)
                    nc.gpsimd.memset(vc3[:, :, 64:66], 0.0)
                return vc3

            vc3 = None
            views = {}

            s3 = src_t[0]
            sEE = s3[:, P_EE:P_EO].rearrange("p (r q) -> p r q", q=66)
            sEO = s3[:, P_EO:P_OE].rearrange("p (r q) -> p r q", q=64)
            sOE = s3[:, P_OE:P_OO].rearrange("p (r q) -> p r q", q=66)
            sOO = s3[:, P_OO:SRCN].rearrange("p (r q) -> p r q", q=64)

            oEE = outp.tile([128, 4096], f16, tag="O", name="oEE")
            oEO = outp.tile([128, 4096], f16, tag="O", name="oEO")
            oOE = outp.tile([128, 4096], f16, tag="O", name="oOE")
            oOO = outp.tile([128, 4096], f16, tag="O", name="oOO")

            def fold_half(slots, vkeys, out_tile, srcv3, h):
                # diag matmuls, PSUM-accumulated 2048-col half;
                # Vector TT multiplies PSUM F~ by the src half directly.
                ft = ps.tile([128, 2048], f32, tag="mm", name="ft")
                f3 = ft.rearrange("p (r q) -> p r q", q=64)
                for i, (sl, vk) in enumerate(zip(slots, vkeys)):
                    v3 = views[vk]
                    for b in range(4):
                        r0 = h * 32 + b * 8
                        nc.tensor.matmul(
                            f3[:, b * 8:b * 8 + 8, :],
                            lhsT=dg[:, sl * 128:sl * 128 + 128],
                            rhs=v3[:, r0:r0 + 8, :],
                            start=(i == 0), stop=(i == len(slots) - 1))
                nc.vector.tensor_tensor(
                    out=out_tile[:, h * 2048:h * 2048 + 2048]
                        .rearrange("p (r q) -> p r q", q=64),
                    in0=f3[:], in1=srcv3[:, h * 32:h * 32 + 32, :],
                    op=MULT)

            for c, csz in enumerate(CHUNKS):
                c0 = coff[c]
                if c == 4:
                    vc3 = emit_vconv()
                    views = dict(v00=vc3[:, 0:64, 0:64],
                                 v10=vc3[:, 1:65, 0:64],
                                 v01=vc3[:, 0:64, 1:65],
                                 v11=vc3[:, 1:65, 1:65])
                pt = ps.tile([128, 2048], f32, tag="mm")
                for kt in range(2):
                    for s0 in range(0, csz, 512):
                        ssz = min(512, csz - s0)
                        nc.tensor.matmul(
                            pt[:, s0:s0 + ssz],
                            lhsT=w_t[kt][:, 0:128],
                            rhs=src_t[kt][:, c0 + s0:c0 + s0 + ssz],
                            start=(kt == 0), stop=(kt == 1))
                nc.scalar.copy(qc[:, c0:c0 + csz], pt[:, 0:csz])
                if pend_s:
                    pend_s.pop(0)()

                if c == EE_RDY:
                    tap_ttr(0, qEE[:, 0:64, 1:65], pr[0])
                    tap_ttr(6, qEE[:, 1:65, 1:65], pr[1])
                    tap_tt(2, qEE[:, 0:64, 2:66], pr[0])
                    tap_tt(8, qEE[:, 1:65, 2:66], pr[1])
                if c == EE_RDY + 2:
                    def fin_ee():
                        exp_group(0, 4)
                    pend_s.append(fin_ee)
                if c == EO_RDY:
                    tap_tt(1, qEO[:, 0:64, 0:64], pr[0])
                    tap_tt(7, qEO[:, 1:65, 0:64], pr[1])
                    for sl in range(4):
                        diag(sl)
                if c == EO_RDY + 2:
                    def fin_eo():
                        exp_group(4, 6)
                    pend_s.append(fin_eo)
                if c == OE_RDY:
                    tap_tt(3, qOE[:, 0:64, 1:65], pr[0])
                    tap_tt(5, qOE[:, 0:64, 2:66], pr[1])
                    diag(4)
                    diag(5)
                if c == 9:
                    # tap4 rows 0:32 == OO chunk 0 exactly
                    tap_ttr(4, qOO[:, 0:32, 0:64], pr[0], sl=24,
                            kview=kv[:, 0:32, :])
                    # oo: F~ = E0*v11 + E2*v10 + E6*v01 + E8*v00
                    fold_half([0, 1, 2, 3], ['v11', 'v10', 'v01', 'v00'],
                              oOO, sEE[:, 1:65, 2:66], 0)
                if c == 10:
                    tap_ttr(4, qOO[:, 32:64, 0:64], pr[1], sl=25,
                            kview=kv[:, 32:64, :])

            while pend_s:
                pend_s.pop(0)()
            exp_group(6, 8)
            diag(6)
            diag(7)

            fold_half([0, 1, 2, 3], ['v11', 'v10', 'v01', 'v00'],
                      oOO, sEE[:, 1:65, 2:66], 1)
            # oe: F~ = E1*v10 + E7*v00
            for h in range(2):
                fold_half([4, 5], ['v10', 'v00'], oOE,
                          sEO[:, 1:65, 0:64], h)

            nc.vector.tensor_tensor(out=sm[:, 8:9], in0=sm[:, 24:25],
                                    in1=sm[:, 25:26], op=ADD)
            exp_group(8, 9)

            # eo: F~ = E3*v01 + E5*v00
            for h in range(2):
                fold_half([6, 7], ['v01', 'v00'], oEO,
                          sOE[:, 0:64, 2:66], h)

            # ---- normalization + tail ----
            nc.vector.tensor_reduce(sm[:, 27:28], sm[:, 9:18],
                                    axis=AX, op=ADD)
            nc.vector.reciprocal(sm[:, 28:29], sm[:, 27:28])
            r = sm[:, 28:29]

            nc.vector.tensor_scalar(out=oOO[:], in0=oOO[:], scalar1=r,
                                    scalar2=None, op0=MULT)
            nc.sync.dma_start(out_d[:, 12288:16384], oOO[:])
            nc.vector.tensor_scalar(out=oOE[:], in0=oOE[:], scalar1=r,
                                    scalar2=None, op0=MULT)
            nc.sync.dma_start(out_d[:, 8192:12288], oOE[:])
            # ee: (E4*r*v00) . sOO
            nc.vector.tensor_scalar(
                out=pr[0][:, 0:4096].rearrange("p (r q) -> p r q", q=64),
                in0=views['v00'], scalar1=sm[:, 17:18], scalar2=r,
                op0=MULT, op1=MULT)
            nc.vector.tensor_tensor(
                out=oEE.rearrange("p (r q) -> p r q", q=64),
                in0=pr[0][:, 0:4096].rearrange("p (r q) -> p r q", q=64),
                in1=sOO[:, 0:64, 0:64], op=MULT)
            nc.sync.dma_start(out_d[:, 0:4096], oEE[:])
            nc.vector.tensor_scalar(out=oEO[:], in0=oEO[:], scalar1=r,
                                    scalar2=None, op0=MULT)
            nc.sync.dma_start(out_d[:, 4096:8192], oEO[:])

    nc.compile()
    return nc


def _get_program(add_bv: bool, per_tap_bias: bool):
    key = (add_bv, per_tap_bias)
    if key not in _prog_cache:
        _prog_cache[key] = _build(add_bv, per_tap_bias)
    return _prog_cache[key]


def _polyphase(x):
    B, C = x.shape[:2]
    ee = np.zeros((B, C, 65, 66), np.float16)
    ee[:, :, :, 1:66] = x[:, :, 0::2, 0::2]
    oe = np.zeros((B, C, 64, 66), np.float16)
    oe[:, :, :, 1:66] = x[:, :, 1::2, 0::2]
    return np.concatenate([
        ee.reshape(B, C, -1),
        x[:, :, 0::2, 1::2].reshape(B, C, -1),
        oe.reshape(B, C, -1),
        x[:, :, 1::2, 1::2].reshape(B, C, -1),
    ], axis=2)


def kernel(feat, src, Wq, bq, Wv, bv):
    from concourse.bass_utils import run_bass_kernel_spmd

    feat = np.asarray(feat, dtype=np.float32)
    src = np.asarray(src, dtype=np.float32)
    Wq = np.asarray(Wq, dtype=np.float32)
    bq = np.asarray(bq, dtype=np.float32)
    Wv = np.asarray(Wv, dtype=np.float32)
    bv = np.asarray(bv, dtype=np.float32)
    B, C, H, W = src.shape
    CH_HALF = C // 2

    src_pad = np.zeros((B, C, 129, 129), np.float16)
    src_pad[:, :, 1:129, 1:129] = src
    src_pk = _polyphase(src_pad)
    feat_pk = np.zeros((B, C, 65, 66), np.float16)
    feat_pk[:, :, 0:64, 0:64] = feat
    feat_pk = feat_pk.reshape(B, C, FEATN)

    add_bv = bool(np.any(bv))
    per_tap_bias = bool(np.any(bq))
    nc = _get_program(add_bv, per_tap_bias)
    ident = np.eye(128, dtype=np.float16)

    in_maps = []
    for core in range(N_CORES):
        b, u = core // 2, core % 2
        own = slice(CH_HALF * u, CH_HALF * u + CH_HALF)
        perm = np.r_[own, slice(CH_HALF * (1 - u), CH_HALF * (1 - u) + CH_HALF)]
        wpack = np.concatenate(
            [Wq[own][:, perm].T, Wv[own][:, perm].T], axis=1
        ).astype(np.float16)
        if per_tap_bias:
            k = feat[b, own].astype(np.float64)
            tot = k.sum((1, 2))
            no_r0 = tot - k[:, 0, :].sum(1)
            no_c0 = tot - k[:, :, 0].sum(1)
            no_rc = no_r0 - k[:, :, 0].sum(1) + k[:, 0, 0]
            sums = [no_rc, no_r0, no_r0, no_c0, tot, tot, no_c0, tot, tot]
            sinit_t = (np.stack(sums, 1) * bq[own, None] * SCALE).astype(
                np.float32)
            sinit = sinit_t[:, SLOT_ORDER]
        else:
            sinit = np.zeros((CH_HALF, 9), np.float32)
        in_maps.append({
            "src": np.ascontiguousarray(src_pk[b, perm]),
            "feat": np.ascontiguousarray(feat_pk[b, perm]),
            "wpack": np.ascontiguousarray(wpack),
            "s_init": sinit,
            "bv": bv[own].reshape(CH_HALF, 1).astype(np.float32),
            "ident": ident,
        })

    res = run_bass_kernel_spmd(nc, in_maps, list(range(N_CORES)),
                               trace=TRACE, **TRACE_KW)
    LAST_RESULT[0] = res

    out = np.empty((B, C, H, W), np.float32)
    for core in range(N_CORES):
        b, u = core // 2, core % 2
        own = slice(CH_HALF * u, CH_HALF * u + CH_HALF)
        r = res.results[core]["out"].astype(np.float32).reshape(
            CH_HALF, 4, 64, 64)
        out[b, own, 0::2, 0::2] = r[:, 0]
        out[b, own, 0::2, 1::2] = r[:, 1]
        out[b, own, 1::2, 0::2] = r[:, 2]
        out[b, own, 1::2, 1::2] = r[:, 3]
    return out
